# revision 9
# baseline (speedup 1.0000x reference)
"""Trainium2 Bass kernel for nn_CrossTransformerDynamicQuery.

Sharding: data-parallel over (batch, query-half): core c handles batch c//2,
query rows [(c%2)*512, (c%2)*512+512). Each core computes its slice of all
three outputs (out, energy, qh) from full inputs; host does pre/post
transposes and the gather.

Device convention: activations live feature-on-partition ("transposed",
[C, rows]); every linear layer is a natural PE matmul. All matmuls run in
float32r (fp32 storage, ~tf32 precision, 1 cycle/row). Attention uses
transposed scores S^T = K Q^T per head, exp on ACT (no max subtraction —
|scale*scores| is small for this problem's distributions), softmax
denominator via ones-matmul; two heads packed per PE pass via row/col
tile_position. The energy output ships as exp(scale*energy) (needed in SBUF
anyway); host recovers energy = log(.)/scale. LayerNorm affine params are
folded host-side into the adjacent weight matrices where possible.
"""

import numpy as np

import concourse.bacc as bacc
import concourse.tile as tile
from concourse import mybir
from concourse.alu_op_type import AluOpType as OP
from concourse.bass_utils import run_bass_kernel_spmd

AF = mybir.ActivationFunctionType
F32R = mybir.dt.float32r
F32 = mybir.dt.float32

B, Q, KLEN, C, H = 4, 1024, 1024, 1024, 16
L = C
CH = L // H          # 64
P = 128
CC = C // P          # 8
R = 512              # local query rows per core
F = 1024             # full batch rows (keys/values)
SCALE = 1.0 / 32.0
N_CORES = 8

_uid = [0]


def _nm(base):
    _uid[0] += 1
    return f"{base}_{_uid[0]}"


def _emit(nc, tc, t, ctxstack):
    sync = nc.sync

    mm = ctxstack.enter_context(tc.tile_pool(name="mm_ps", bufs=4, space="PSUM"))
    acc = ctxstack.enter_context(tc.tile_pool(name="acc_ps", bufs=2, space="PSUM"))
    den = ctxstack.enter_context(tc.tile_pool(name="den_ps", bufs=2, space="PSUM"))

    const = ctxstack.enter_context(tc.tile_pool(name="const", bufs=1))
    ones128 = const.tile([P, 1], F32R, name="ones128")
    sync.dma_start(out=ones128, in_=t["onesv"].rearrange("(p one) -> p one",
                                                         one=1))
    onesrow = const.tile([1, P], F32R, name="onesrow")
    sync.dma_start(out=onesrow, in_=t["onesv"].rearrange("(one p) -> one p",
                                                         one=1))
    eps1 = const.tile([1, 1], F32, name="eps1")
    nc.vector.memset(eps1, 1e-5)

    def vec_tile(name, nch):
        tl = const.tile([P, nch], F32, name=name + "_t")
        sync.dma_start(out=tl, in_=t[name].rearrange("(mc p) -> p mc", p=P))
        return tl

    cq8 = vec_tile("cq", 8)
    ck8 = vec_tile("ck", 8)
    bo8 = vec_tile("bo", 8)
    brz16 = vec_tile("brz", 16)
    bin8 = vec_tile("bin", 8)
    bhn8 = vec_tile("bhn", 8)
    bco8 = vec_tile("bco", 8)
    ln1w8 = vec_tile("ln1w", 8)
    ln1b8 = vec_tile("ln1b", 8)
    ln2w16 = vec_tile("ln2w", 16)
    ln2b16 = vec_tile("ln2b", 16)

    def load_T(pool, dram, nch, width, name, bufs=None, tag=""):
        tl = pool.tile([P, nch, width], F32R, name=name, bufs=bufs, tag=tag)
        src = dram.rearrange("(ko ki) r -> ki ko r", ki=P)
        for k in range(nch):
            sync.dma_start(out=tl[:, k], in_=src[:, k])
        return tl

    # ---------- LayerNorm (transposed layout) ----------
    def ln_stats(xt, nch, W, tag, sqp, statp, bcp):
        nfeat = float(nch * P)
        s = statp.tile([1, W], F32, name=_nm(f"s_{tag}"), tag="stat")
        ss = statp.tile([1, W], F32, name=_nm(f"ss_{tag}"), tag="stat")
        for n in range(W // 512):
            nsl = slice(n * 512, (n + 1) * 512)
            ps = den.tile([33, 512], F32, name=_nm("ps_s"), tag="den")
            for k in range(nch):
                nc.tensor.matmul(ps[0:1], ones128, xt[:, k, nsl],
                                 start=(k == 0), stop=(k == nch - 1))
            nc.vector.tensor_copy(out=s[:, nsl], in_=ps[0:1])
            ps2 = den.tile([33, 512], F32, name=_nm("ps_ss"), tag="den")
            for k in range(nch):
                sq = sqp.tile([P, 512], F32R, name=_nm("sq"), tag="sq")
                nc.scalar.activation(out=sq, in_=xt[:, k, nsl], func=AF.Square)
                nc.tensor.matmul(ps2[0:1], ones128, sq,
                                 start=(k == 0), stop=(k == nch - 1))
            nc.vector.tensor_copy(out=ss[:, nsl], in_=ps2[0:1])
        # s -> mean, ss -> E[x^2] (in place)
        nc.scalar.mul(out=s, in_=s, mul=1.0 / nfeat)
        nc.scalar.mul(out=ss, in_=ss, mul=1.0 / nfeat)
        m2 = statp.tile([1, W], F32, name=_nm(f"m2_{tag}"), tag="m2", bufs=1)
        nc.vector.tensor_mul(out=m2, in0=s, in1=s)
        nc.vector.tensor_sub(out=ss, in0=ss, in1=m2)          # ss = var
        nc.scalar.activation(out=ss, in_=ss, func=AF.Sqrt, bias=eps1)
        nc.vector.reciprocal(out=m2, in_=ss)                  # m2 = rstd
        mean_r = statp.tile([1, W], F32R, name=_nm(f"mr_{tag}"), tag="statr")
        nc.vector.tensor_copy(out=mean_r, in_=s)
        rstd_r = statp.tile([1, W], F32R, name=_nm(f"rr_{tag}"), tag="statr")
        nc.vector.tensor_copy(out=rstd_r, in_=m2)
        mb = bcp.tile([P, W], F32, name=_nm(f"mb_{tag}"), tag="mb")
        rb = bcp.tile([P, W], F32, name=_nm(f"rb_{tag}"), tag="rb")
        for n in range(W // 512):
            nsl = slice(n * 512, (n + 1) * 512)
            for src, dst in ((mean_r, mb), (rstd_r, rb)):
                pb = mm.tile([P, 512], F32, name=_nm("ps_bc"), tag="mm")
                nc.tensor.matmul(pb, onesrow, src[:, nsl], start=True,
                                 stop=True)
                nc.vector.tensor_copy(out=dst[:, nsl], in_=pb)
        return mb, rb

    def ln_normalize(xt, nch, mb, rb):
        for k in range(nch):
            nc.vector.tensor_sub(out=xt[:, k], in0=xt[:, k], in1=mb)
            nc.vector.tensor_mul(out=xt[:, k], in0=xt[:, k], in1=rb)

    # ---------- transposed projection ----------
    def proj_T(wdram, nk, nm, rhs_of, W, out_cb, wpool, gcols, wtag,
               bias=None, act=AF.Copy):
        """out[m] = sum_k W[k*128:(k+1)*128, m*128:(m+1)*128].T @ rhs_of(k).
        out_cb(m, nsl, ps) consumes the psum."""
        mper = gcols // P
        for g in range((nm * P) // gcols):
            wt = wpool.tile([P, nk, gcols], F32R, name=_nm("wt"), tag=wtag)
            for k in range(nk):
                sync.dma_start(
                    out=wt[:, k],
                    in_=wdram[k * P:(k + 1) * P, g * gcols:(g + 1) * gcols])
            for mi in range(mper):
                m = g * mper + mi
                for n in range(W // 512):
                    nsl = slice(n * 512, (n + 1) * 512)
                    ps = mm.tile([P, 512], F32, name=_nm("ps_pj"), tag="mm")
                    for k in range(nk):
                        nc.tensor.matmul(ps, wt[:, k, mi * P:(mi + 1) * P],
                                         rhs_of(k)[:, nsl],
                                         start=(k == 0), stop=(k == nk - 1))
                    out_cb(m, nsl, ps)

    def copy_out(out_tile, bias=None):
        def cb(m, nsl, ps):
            if bias is None:
                nc.scalar.copy(out=out_tile[:, m, nsl], in_=ps)
            else:
                nc.vector.tensor_scalar(
                    out=out_tile[:, m, nsl], in0=ps,
                    scalar1=bias[:, m:m + 1], scalar2=None, op0=OP.add)
        return cb

    # ---------- attention head-pair loop ----------
    def attn_heads(kTt, qrhs, vRt, oT, expA_p, expB_p, rbp, eexp_dram=None):
        FCH = F // P
        for h2 in range(8):
            expA = expA_p.tile([P, FCH, 512], F32R, name=_nm("expA"),
                               tag="expA")
            expB = expB_p.tile([P, FCH, 512], F32R, name=_nm("expB"),
                               tag="expB")
            for fc in range(FCH):
                psA = mm.tile([P, 512], F32, name=_nm("ps_sA"), tag="mm")
                nc.tensor.matmul(psA, kTt[0:CH, h2, fc * P:(fc + 1) * P],
                                 qrhs[0:CH, h2, :], start=True, stop=True)
                nc.scalar.activation(out=expA[:, fc], in_=psA, func=AF.Exp,
                                     scale=SCALE)
                psB = mm.tile([P, 512], F32, name=_nm("ps_sB"), tag="mm")
                nc.tensor.matmul(psB, kTt[CH:P, h2, fc * P:(fc + 1) * P],
                                 qrhs[CH:P, h2, :], start=True, stop=True)
                nc.scalar.activation(out=expB[:, fc], in_=psB, func=AF.Exp,
                                     scale=SCALE)
                if eexp_dram is not None:
                    sync.dma_start(
                        out=eexp_dram[2 * h2, fc * P:(fc + 1) * P, :],
                        in_=expA[:, fc])
                    sync.dma_start(
                        out=eexp_dram[2 * h2 + 1, fc * P:(fc + 1) * P, :],
                        in_=expB[:, fc])
            psdA = den.tile([1, 512], F32, name=_nm("ps_dA"), tag="den")
            for fc in range(FCH):
                nc.tensor.matmul(psdA, ones128, expA[:, fc],
                                 start=(fc == 0), stop=(fc == FCH - 1))
            psdB = den.tile([1, 512], F32, name=_nm("ps_dB"), tag="den")
            for fc in range(FCH):
                nc.tensor.matmul(psdB, ones128, expB[:, fc],
                                 start=(fc == 0), stop=(fc == FCH - 1))
            psoA = acc.tile([CH, 512], F32, name=_nm("ps_oA"), tag="acc")
            for fc in range(FCH):
                nc.tensor.matmul(
                    psoA, vRt[:, fc, (2 * h2) * CH:(2 * h2 + 1) * CH],
                    expA[:, fc], start=(fc == 0), stop=(fc == FCH - 1))
            psoB = acc.tile([CH, 512], F32, name=_nm("ps_oB"), tag="acc")
            for fc in range(FCH):
                nc.tensor.matmul(
                    psoB, vRt[:, fc, (2 * h2 + 1) * CH:(2 * h2 + 2) * CH],
                    expB[:, fc], start=(fc == 0), stop=(fc == FCH - 1))
            for side, psd, pso in (("A", psdA, psoA), ("B", psdB, psoB)):
                rd = rbp.tile([1, 512], F32, name=_nm("rd"), tag="rd")
                nc.vector.reciprocal(out=rd, in_=psd)
                rdr = rbp.tile([1, 512], F32R, name=_nm("rdr"), tag="rdr")
                nc.vector.tensor_copy(out=rdr, in_=rd)
                psrb = mm.tile([P, 512], F32, name=_nm("ps_rb"), tag="mm")
                nc.tensor.matmul(psrb[0:CH], onesrow[:, 0:CH], rdr,
                                 start=True, stop=True)
                rb64 = rbp.tile([CH, 512], F32, name=_nm("rb64"), tag="rb64")
                nc.vector.tensor_copy(out=rb64, in_=psrb[0:CH])
                if side == "A":
                    nc.vector.tensor_mul(out=oT[0:CH, h2], in0=pso, in1=rb64)
                else:
                    tmpB = rbp.tile([CH, 512], F32R, name=_nm("tmpB"),
                                    tag="tmpB")
                    nc.vector.tensor_mul(out=tmpB, in0=pso, in1=rb64)
                    sync.dma_start(out=oT[CH:P, h2], in_=tmpB)

    # carriers that cross phase boundaries
    x1T = ctxstack.enter_context(tc.tile_pool(name="x1p", bufs=1)) \
        .tile([P, CC, R], F32R, name="x1T")

    # ============================ SA block ============================
    with tc.tile_pool(name="qkv", bufs=1) as qkv:
        kT = qkv.tile([P, CC, F], F32R, name="kT")
        qTt = qkv.tile([P, CC, R], F32R, name="qTt")
        vR = qkv.tile([P, CC, F], F32R, name="vR")

        with tc.tile_pool(name="saTmp", bufs=1) as saT, \
             tc.tile_pool(name="saW", bufs=2) as saW, \
             tc.tile_pool(name="saSq", bufs=2) as saSq, \
             tc.tile_pool(name="saStat", bufs=2) as saStat, \
             tc.tile_pool(name="saBc", bufs=1) as saBc:
            qTf = load_T(saT, t["qT"], CC, F, "qTf", tag="qTf")
            mbF, rbF = ln_stats(qTf, CC, F, "saF", saSq, saStat, saBc)
            ln_normalize(qTf, CC, mbF, rbF)
            proj_T(t["wk"], CC, CC, lambda k: qTf[:, k], F,
                   copy_out(kT, ck8), saW, 256, "w")
            # V row-major: lhsT = xn chunks, rhs = wv halves
            for n in range(2):
                nsl = slice(n * 512, (n + 1) * 512)
                wvt = saW.tile([P, CC, 512], F32R, name=_nm("wvt"), tag="wv",
                               bufs=1)
                for k in range(CC):
                    sync.dma_start(out=wvt[:, k],
                                   in_=t["wv"][k * P:(k + 1) * P, nsl])
                for fm in range(CC):
                    ps = mm.tile([P, 512], F32, name=_nm("ps_v"), tag="mm")
                    for k in range(CC):
                        nc.tensor.matmul(ps, qTf[:, k, fm * P:(fm + 1) * P],
                                         wvt[:, k],
                                         start=(k == 0), stop=(k == CC - 1))
                    nc.scalar.copy(out=vR[:, fm, nsl], in_=ps)
            qTl = load_T(saT, t["qTloc"], CC, R, "qTl", tag="qTf")
            mbL, rbL = ln_stats(qTl, CC, R, "saL", saSq, saStat, saBc)
            ln_normalize(qTl, CC, mbL, rbL)
            proj_T(t["wq"], CC, CC, lambda k: qTl[:, k], R,
                   copy_out(qTt, cq8), saW, 256, "w")

        with tc.tile_pool(name="saA", bufs=1) as expA_p, \
             tc.tile_pool(name="saB", bufs=1) as expB_p, \
             tc.tile_pool(name="saRb", bufs=2) as saRb, \
             tc.tile_pool(name="saO", bufs=1) as saO, \
             tc.tile_pool(name="saW2", bufs=2) as saW2, \
             tc.tile_pool(name="saRes", bufs=2) as saRes:
            oT = saO.tile([P, CC, R], F32R, name="oT")
            attn_heads(kT, qTt, vR, oT, expA_p, expB_p, saRb)

            qlr = t["qTloc"].rearrange("(ko ki) r -> ki ko r", ki=P)

            def sa_out(m, nsl, ps):
                res = saRes.tile([P, R], F32R, name=_nm("res"), tag="res")
                sync.dma_start(out=res, in_=qlr[:, m])
                nc.vector.scalar_tensor_tensor(
                    out=x1T[:, m], in0=ps, scalar=bo8[:, m:m + 1], in1=res,
                    op0=OP.add, op1=OP.add)

            proj_T(t["wo"], CC, CC, lambda k: oT[:, k], R, sa_out,
                   saW2, 256, "wo")

    # ============================ GRU block ============================
    hpT = ctxstack.enter_context(tc.tile_pool(name="hpp", bufs=1)) \
        .tile([P, CC, R], F32R, name="hpT")
    with tc.tile_pool(name="gruT", bufs=1) as gT, \
         tc.tile_pool(name="gruW", bufs=2) as gW, \
         tc.tile_pool(name="gruSq", bufs=2) as gSq, \
         tc.tile_pool(name="gruStat", bufs=2) as gStat, \
         tc.tile_pool(name="gruBc", bufs=1) as gBc, \
         tc.tile_pool(name="gruTmp", bufs=2) as gTmp:
        hT = load_T(gT, t["hT"], CC, R, "hTt")
        xn2 = gT.tile([P, CC, R], F32R, name="xn2")
        for k in range(CC):
            nc.vector.tensor_copy(out=xn2[:, k], in_=x1T[:, k])
        mb2, rb2 = ln_stats(xn2, CC, R, "gru", gSq, gStat, gBc)
        ln_normalize(xn2, CC, mb2, rb2)

        rT = gT.tile([P, CC, R], F32R, name="rT")
        zT = gT.tile([P, CC, R], F32R, name="zT")
        for g in range(8):
            wt = gW.tile([P, 16, 256], F32R, name=_nm("wrzt"), tag="wrz")
            for k in range(16):
                sync.dma_start(out=wt[:, k],
                               in_=t["wrz"][k * P:(k + 1) * P,
                                            g * 256:(g + 1) * 256])
            for mi in range(2):
                m = g * 2 + mi
                ps = mm.tile([P, 512], F32, name=_nm("ps_rz"), tag="mm")
                for k in range(16):
                    rhs = xn2[:, k, :] if k < CC else hT[:, k - CC, :]
                    nc.tensor.matmul(ps, wt[:, k, mi * P:(mi + 1) * P], rhs,
                                     start=(k == 0), stop=(k == 15))
                dst = rT[:, m] if m < CC else zT[:, m - CC]
                nc.scalar.activation(out=dst, in_=ps, func=AF.Sigmoid,
                                     bias=brz16[:, m:m + 1])
        ghn = gT.tile([P, CC, R], F32R, name="ghn")
        proj_T(t["whn"], CC, CC, lambda k: hT[:, k], R,
               copy_out(ghn, bhn8), gW, 256, "wg")
        nT = rT  # n overwrites r per chunk (r is consumed first)

        def gin_out(m, nsl, ps):
            tmp = gTmp.tile([P, R], F32R, name=_nm("rghn"), tag="rghn")
            nc.vector.tensor_mul(out=tmp, in0=rT[:, m], in1=ghn[:, m])
            nc.vector.tensor_tensor(out=nT[:, m], in0=ps, in1=tmp, op=OP.add)
            nc.scalar.activation(out=nT[:, m], in_=nT[:, m], func=AF.Tanh,
                                 bias=bin8[:, m:m + 1])

        proj_T(t["win"], CC, CC, lambda k: xn2[:, k], R, gin_out,
               gW, 256, "wg")
        for m in range(CC):
            d = gTmp.tile([P, R], F32R, name=_nm("hmn"), tag="rghn")
            nc.vector.tensor_sub(out=d, in0=hT[:, m], in1=nT[:, m])
            nc.vector.tensor_mul(out=d, in0=zT[:, m], in1=d)
            nc.vector.tensor_add(out=hpT[:, m], in0=nT[:, m], in1=d)
            sync.dma_start(out=t["qhT"][m * P:(m + 1) * P, :], in_=hpT[:, m])

    # ============================ CA block ============================
    x2T = ctxstack.enter_context(tc.tile_pool(name="x2p", bufs=1)) \
        .tile([P, CC, R], F32R, name="x2T")
    with tc.tile_pool(name="caM", bufs=1) as caM:
        kkT = caM.tile([P, CC, F], F32R, name="kkT")
        v2R = caM.tile([P, CC, F], F32R, name="v2R")
        with tc.tile_pool(name="caCtx", bufs=1) as caC, \
             tc.tile_pool(name="caW", bufs=2) as caW:
            ctxT = load_T(caC, t["ctxT"], CC, F, "ctxTt")
            proj_T(t["wck"], CC, CC, lambda k: ctxT[:, k], F,
                   copy_out(kkT), caW, 256, "w")
            for n in range(2):
                nsl = slice(n * 512, (n + 1) * 512)
                wcvt = caW.tile([P, CC, 512], F32R, name=_nm("wcvt"),
                                tag="wv", bufs=1)
                for k in range(CC):
                    sync.dma_start(out=wcvt[:, k],
                                   in_=t["wcv"][k * P:(k + 1) * P, nsl])
                for fm in range(CC):
                    ps = mm.tile([P, 512], F32, name=_nm("ps_cv"), tag="mm")
                    for k in range(CC):
                        nc.tensor.matmul(ps, ctxT[:, k, fm * P:(fm + 1) * P],
                                         wcvt[:, k],
                                         start=(k == 0), stop=(k == CC - 1))
                    nc.scalar.copy(out=v2R[:, fm, nsl], in_=ps)

        with tc.tile_pool(name="caA", bufs=1) as cexpA, \
             tc.tile_pool(name="caB", bufs=1) as cexpB, \
             tc.tile_pool(name="caRb", bufs=2) as caRb, \
             tc.tile_pool(name="caO", bufs=1) as caO, \
             tc.tile_pool(name="caW2", bufs=2) as caW2:
            oT2 = caO.tile([P, CC, R], F32R, name="oT2")
            attn_heads(kkT, hpT, v2R, oT2, cexpA, cexpB, caRb,
                       eexp_dram=t["eexpT"])

            def ca_out(m, nsl, ps):
                nc.vector.scalar_tensor_tensor(
                    out=x2T[:, m], in0=ps, scalar=bco8[:, m:m + 1],
                    in1=x1T[:, m], op0=OP.add, op1=OP.add)

            proj_T(t["wco"], CC, CC, lambda k: oT2[:, k], R, ca_out,
                   caW2, 256, "wco")

    # ============================ MLP block ============================
    with tc.tile_pool(name="mlpT", bufs=1) as mT, \
         tc.tile_pool(name="mlpW", bufs=2) as mW, \
         tc.tile_pool(name="mlpSq", bufs=2) as mSq, \
         tc.tile_pool(name="mlpStat", bufs=2) as mStat, \
         tc.tile_pool(name="mlpBc", bufs=1) as mBc:
        s3 = mT.tile([P, CC, R], F32R, name="s3")
        for k in range(CC):
            nc.vector.tensor_copy(out=s3[:, k], in_=x2T[:, k])
        mb3, rb3 = ln_stats(s3, CC, R, "ln1", mSq, mStat, mBc)
        ln_normalize(s3, CC, mb3, rb3)
        for k in range(CC):
            nc.scalar.activation(out=s3[:, k], in_=s3[:, k], func=AF.Silu,
                                 scale=ln1w8[:, k:k + 1],
                                 bias=ln1b8[:, k:k + 1])
        m1 = mT.tile([P, 16, R], F32R, name="m1")
        proj_T(t["w1"], CC, 16, lambda k: s3[:, k], R,
               copy_out(m1), mW, 256, "w1")
        mb4, rb4 = ln_stats(m1, 16, R, "ln2", mSq, mStat, mBc)
        ln_normalize(m1, 16, mb4, rb4)
        for k in range(16):
            nc.scalar.activation(out=m1[:, k], in_=m1[:, k], func=AF.Silu,
                                 scale=ln2w16[:, k:k + 1],
                                 bias=ln2b16[:, k:k + 1])

        def mlp_out(m, nsl, ps):
            ot = mT.tile([P, R], F32R, name=_nm("otf"), tag="otf")
            nc.vector.tensor_tensor(out=ot, in0=ps, in1=x2T[:, m], op=OP.add)
            sync.dma_start(out=t["outT"][m * P:(m + 1) * P, :], in_=ot)

        proj_T(t["w2"], 16, CC, lambda k: m1[:, k], R, mlp_out,
               mW, 128, "w2")


def build_nc():
    from contextlib import ExitStack
    nc = bacc.Bacc("TRN2", target_bir_lowering=False, debug=False)
    t = {}

    def inp(name, shape):
        t[name] = nc.dram_tensor(name, list(shape), F32R, kind="ExternalInput")

    inp("onesv", (P,))
    inp("qT", (C, F)); inp("qTloc", (C, R)); inp("hT", (L, R))
    inp("ctxT", (C, F))
    inp("wq", (C, L)); inp("wk", (C, L)); inp("wv", (C, C)); inp("wo", (C, C))
    inp("wrz", (2 * C, 2 * L)); inp("win", (C, L)); inp("whn", (L, L))
    inp("wck", (C, L)); inp("wcv", (C, C)); inp("wco", (C, C))
    inp("w1", (C, 2 * C)); inp("w2", (2 * C, C))
    for name, n in [("cq", C), ("ck", C), ("bo", C), ("brz", 2 * L),
                    ("bin", L), ("bhn", L), ("bco", C),
                    ("ln1w", C), ("ln1b", C), ("ln2w", 2 * C),
                    ("ln2b", 2 * C)]:
        t[name] = nc.dram_tensor(name, [n], F32, kind="ExternalInput")
    t["outT"] = nc.dram_tensor("outT", [C, R], F32R, kind="ExternalOutput")
    t["qhT"] = nc.dram_tensor("qhT", [L, R], F32R, kind="ExternalOutput")
    t["eexpT"] = nc.dram_tensor("eexpT", [H, KLEN, R], F32R,
                                kind="ExternalOutput")

    with tile.TileContext(nc) as tc, ExitStack() as ctxstack:
        _emit(nc, tc, t, ctxstack)
    nc.compile()
    return nc


def prep_inputs(inputs):
    f = np.float32
    g = {k: np.asarray(v, f) for k, v in inputs.items()}
    qkvw = g["sa_qkv_w"].reshape(C, H, 2 * CH + C // H)
    Wq = np.ascontiguousarray(qkvw[:, :, :CH].reshape(C, L))
    Wk = np.ascontiguousarray(qkvw[:, :, CH:2 * CH].reshape(C, L))
    Wv = np.ascontiguousarray(qkvw[:, :, 2 * CH:].reshape(C, C))
    wln, bln = g["sa_norm_w"], g["sa_norm_b"]
    wq = wln[:, None] * Wq
    cq = bln @ Wq
    wk = wln[:, None] * Wk
    ck = bln @ Wk
    wv = wln[:, None] * Wv
    cv = bln @ Wv
    wo = g["sa_out_w"]
    bo = g["sa_out_b"] + cv @ wo

    W_ih, W_hh = g["gru_w_ih"], g["gru_w_hh"]
    b_ih, b_hh = g["gru_b_ih"], g["gru_b_hh"]
    cw, cb = g["ca_norm_w"], g["ca_norm_b"]
    ih_rzT = W_ih[:2 * L].T
    hh_rzT = W_hh[:2 * L].T
    wrz = np.concatenate([cw[:, None] * ih_rzT, hh_rzT], axis=0)
    brz = b_ih[:2 * L] + b_hh[:2 * L] + cb @ ih_rzT
    in_T = W_ih[2 * L:].T
    win = cw[:, None] * in_T
    bin_ = b_ih[2 * L:] + cb @ in_T
    whn = np.ascontiguousarray(W_hh[2 * L:].T)
    bhn = b_hh[2 * L:]

    shared = {
        "wq": wq, "wk": wk, "wv": wv, "wo": wo,
        "cq": cq, "ck": ck, "bo": bo,
        "wrz": wrz, "brz": brz, "win": win, "bin": bin_,
        "whn": whn, "bhn": bhn,
        "wck": g["ca_k_w"], "wcv": g["ca_v_w"], "wco": g["ca_out_w"],
        "bco": g["ca_out_b"],
        "w1": g["lin1_w"], "w2": g["lin2_w"],
        "ln1w": g["ln1_w"], "ln1b": g["ln1_b"],
        "ln2w": g["ln2_w"], "ln2b": g["ln2_b"],
    }
    shared = {k: np.ascontiguousarray(v, f) for k, v in shared.items()}

    in_maps = []
    for c in range(N_CORES):
        b = c // 2
        r0 = (c % 2) * R
        d = dict(shared)
        d["onesv"] = np.ones(128, np.float32)
        d["qT"] = np.ascontiguousarray(g["queries"][b].T)
        d["qTloc"] = np.ascontiguousarray(g["queries"][b, r0:r0 + R].T)
        d["hT"] = np.ascontiguousarray(
            g["queries_hidden"][b, r0:r0 + R].reshape(R, L).T)
        d["ctxT"] = np.ascontiguousarray(g["contexts"][b].T)
        in_maps.append(d)
    return in_maps


def unshard(results):
    out = np.empty((B, Q, C), np.float32)
    qh = np.empty((B, Q, H, CH), np.float32)
    energy = np.empty((B, H, Q, KLEN), np.float32)
    for c in range(N_CORES):
        b = c // 2
        r0 = (c % 2) * R
        res = results[c]
        out[b, r0:r0 + R] = res["outT"].T
        qh[b, r0:r0 + R] = res["qhT"].T.reshape(R, H, CH)
        with np.errstate(divide="ignore"):
            e = np.log(res["eexpT"].astype(np.float64)) / SCALE
        energy[b, :, r0:r0 + R, :] = e.transpose(0, 2, 1).astype(np.float32)
    return out, energy, qh


_NC_CACHE = None


def kernel(**inputs):
    global _NC_CACHE
    if _NC_CACHE is None:
        _NC_CACHE = build_nc()
    in_maps = prep_inputs(inputs)
    res = run_bass_kernel_spmd(_NC_CACHE, in_maps,
                               core_ids=list(range(N_CORES)))
    return unshard(res.results)


# revision 11
# speedup vs baseline: 1.1084x; 1.1084x over previous
"""Trainium2 Bass kernel for nn_CrossTransformerDynamicQuery.

Sharding: data-parallel over (batch, query-half): core c handles batch c//2,
query rows [(c%2)*512, (c%2)*512+512). Each core computes its slice of all
three outputs (out, energy, qh) from full inputs; host does pre/post
transposes and the gather.

Device convention: activations live feature-on-partition ("transposed",
[C, rows]); every linear layer is a natural PE matmul. All matmuls run in
float32r (fp32 storage, ~tf32 precision, 1 cycle/row). Attention uses
transposed scores S^T = K Q^T per head, exp on ACT (no max subtraction —
|scale*scores| is small for this problem's distributions), softmax
denominator via ones-matmul; two heads packed per PE pass via row/col
tile_position. The energy output ships as exp(scale*energy) (needed in SBUF
anyway); host recovers energy = log(.)/scale. LayerNorm affine params are
folded host-side into the adjacent weight matrices where possible.
"""

import numpy as np

import concourse.bacc as bacc
import concourse.tile as tile
from concourse import mybir
from concourse.alu_op_type import AluOpType as OP
from concourse.bass_utils import run_bass_kernel_spmd

AF = mybir.ActivationFunctionType
F32R = mybir.dt.float32r
F32 = mybir.dt.float32

B, Q, KLEN, C, H = 4, 1024, 1024, 1024, 16
L = C
CH = L // H          # 64
P = 128
CC = C // P          # 8
R = 512              # local query rows per core
F = 1024             # full batch rows (keys/values)
SCALE = 1.0 / 32.0
N_CORES = 8

_uid = [0]


def _nm(base):
    _uid[0] += 1
    return f"{base}_{_uid[0]}"


def _emit(nc, tc, t, ctxstack, phases=4):
    sync = nc.sync

    mm = ctxstack.enter_context(tc.tile_pool(name="mm_ps", bufs=4, space="PSUM"))
    acc = ctxstack.enter_context(tc.tile_pool(name="acc_ps", bufs=2, space="PSUM"))
    den = ctxstack.enter_context(tc.tile_pool(name="den_ps", bufs=2, space="PSUM"))

    const = ctxstack.enter_context(tc.tile_pool(name="const", bufs=1))
    ones128 = const.tile([P, 1], F32R, name="ones128")
    sync.dma_start(out=ones128, in_=t["onesv"].rearrange("(p one) -> p one",
                                                         one=1))
    onesrow = const.tile([1, P], F32R, name="onesrow")
    sync.dma_start(out=onesrow, in_=t["onesv"].rearrange("(one p) -> one p",
                                                         one=1))
    eps1 = const.tile([1, 1], F32, name="eps1")
    nc.vector.memset(eps1, 1e-5)

    def vec_tile(name, nch):
        tl = const.tile([P, nch], F32, name=name + "_t")
        sync.dma_start(out=tl, in_=t[name].rearrange("(mc p) -> p mc", p=P))
        return tl

    cq8 = vec_tile("cq", 8)
    ck8 = vec_tile("ck", 8)
    bo8 = vec_tile("bo", 8)
    brz16 = vec_tile("brz", 16)
    bin8 = vec_tile("bin", 8)
    bhn8 = vec_tile("bhn", 8)
    bco8 = vec_tile("bco", 8)
    ln1w8 = vec_tile("ln1w", 8)
    ln1b8 = vec_tile("ln1b", 8)
    ln2w16 = vec_tile("ln2w", 16)
    ln2b16 = vec_tile("ln2b", 16)

    def load_T(pool, dram, nch, width, name, bufs=None, tag=""):
        tl = pool.tile([P, nch, width], F32R, name=name, bufs=bufs, tag=tag)
        src = dram.rearrange("(ko ki) r -> ki ko r", ki=P)
        for k in range(nch):
            sync.dma_start(out=tl[:, k], in_=src[:, k])
        return tl

    # ---------- LayerNorm (transposed layout) ----------
    def ln_stats(xt, nch, W, tag, sqp, statp, bcp):
        nfeat = float(nch * P)
        s = statp.tile([1, W], F32, name=_nm(f"s_{tag}"), tag="stat")
        ss = statp.tile([1, W], F32, name=_nm(f"ss_{tag}"), tag="stat")
        for n in range(W // 512):
            nsl = slice(n * 512, (n + 1) * 512)
            ps = den.tile([33, 512], F32, name=_nm("ps_s"), tag="den")
            for k in range(nch):
                nc.tensor.matmul(ps[0:1], ones128, xt[:, k, nsl],
                                 start=(k == 0), stop=(k == nch - 1))
            nc.vector.tensor_copy(out=s[:, nsl], in_=ps[0:1])
            ps2 = den.tile([33, 512], F32, name=_nm("ps_ss"), tag="den")
            for k in range(nch):
                sq = sqp.tile([P, 512], F32R, name=_nm("sq"), tag="sq")
                nc.scalar.activation(out=sq, in_=xt[:, k, nsl], func=AF.Square)
                nc.tensor.matmul(ps2[0:1], ones128, sq,
                                 start=(k == 0), stop=(k == nch - 1))
            nc.vector.tensor_copy(out=ss[:, nsl], in_=ps2[0:1])
        # s -> mean, ss -> E[x^2] (in place)
        nc.scalar.mul(out=s, in_=s, mul=1.0 / nfeat)
        nc.scalar.mul(out=ss, in_=ss, mul=1.0 / nfeat)
        m2 = statp.tile([1, W], F32, name=_nm(f"m2_{tag}"), tag="m2", bufs=1)
        nc.vector.tensor_mul(out=m2, in0=s, in1=s)
        nc.vector.tensor_sub(out=ss, in0=ss, in1=m2)          # ss = var
        nc.scalar.activation(out=ss, in_=ss, func=AF.Sqrt, bias=eps1)
        nc.vector.reciprocal(out=m2, in_=ss)                  # m2 = rstd
        mean_r = statp.tile([1, W], F32R, name=_nm(f"mr_{tag}"), tag="statr")
        nc.vector.tensor_copy(out=mean_r, in_=s)
        rstd_r = statp.tile([1, W], F32R, name=_nm(f"rr_{tag}"), tag="statr")
        nc.vector.tensor_copy(out=rstd_r, in_=m2)
        mb = bcp.tile([P, W], F32, name=_nm(f"mb_{tag}"), tag="mb")
        rb = bcp.tile([P, W], F32, name=_nm(f"rb_{tag}"), tag="rb")
        for n in range(W // 512):
            nsl = slice(n * 512, (n + 1) * 512)
            for src, dst in ((mean_r, mb), (rstd_r, rb)):
                pb = mm.tile([P, 512], F32, name=_nm("ps_bc"), tag="mm")
                nc.tensor.matmul(pb, onesrow, src[:, nsl], start=True,
                                 stop=True)
                nc.vector.tensor_copy(out=dst[:, nsl], in_=pb)
        return mb, rb

    def ln_normalize(xt, nch, mb, rb):
        for k in range(nch):
            nc.vector.tensor_sub(out=xt[:, k], in0=xt[:, k], in1=mb)
            nc.vector.tensor_mul(out=xt[:, k], in0=xt[:, k], in1=rb)

    # ---------- transposed projection ----------
    def proj_T(wdram, nk, nm, rhs_of, W, out_cb, wpool, gcols, wtag,
               bias=None, act=AF.Copy):
        """out[m] = sum_k W[k*128:(k+1)*128, m*128:(m+1)*128].T @ rhs_of(k).
        out_cb(m, nsl, ps) consumes the psum."""
        mper = gcols // P
        for g in range((nm * P) // gcols):
            wt = wpool.tile([P, nk, gcols], F32R, name=_nm("wt"), tag=wtag)
            for k in range(nk):
                sync.dma_start(
                    out=wt[:, k],
                    in_=wdram[k * P:(k + 1) * P, g * gcols:(g + 1) * gcols])
            for mi in range(mper):
                m = g * mper + mi
                for n in range(W // 512):
                    nsl = slice(n * 512, (n + 1) * 512)
                    ps = mm.tile([P, 512], F32, name=_nm("ps_pj"), tag="mm")
                    for k in range(nk):
                        nc.tensor.matmul(ps, wt[:, k, mi * P:(mi + 1) * P],
                                         rhs_of(k)[:, nsl],
                                         start=(k == 0), stop=(k == nk - 1))
                    out_cb(m, nsl, ps)

    def copy_out(out_tile, bias=None):
        def cb(m, nsl, ps):
            if bias is None:
                nc.scalar.copy(out=out_tile[:, m, nsl], in_=ps)
            else:
                nc.vector.tensor_scalar(
                    out=out_tile[:, m, nsl], in0=ps,
                    scalar1=bias[:, m:m + 1], scalar2=None, op0=OP.add)
        return cb

    # ---------- attention head-pair loop ----------
    def attn_heads(kTt, qrhs, vRt, oT, expA_p, expB_p, rbp, eexp_dram=None):
        FCH = F // P
        for h2 in range(8):
            expA = expA_p.tile([P, FCH, 512], F32R, name=_nm("expA"),
                               tag="expA")
            expB = expB_p.tile([P, FCH, 512], F32R, name=_nm("expB"),
                               tag="expB")
            for fc in range(FCH):
                psA = mm.tile([P, 512], F32, name=_nm("ps_sA"), tag="mm")
                nc.tensor.matmul(psA, kTt[0:CH, h2, fc * P:(fc + 1) * P],
                                 qrhs[0:CH, h2, :], start=True, stop=True)
                nc.scalar.activation(out=expA[:, fc], in_=psA, func=AF.Exp,
                                     scale=SCALE)
                psB = mm.tile([P, 512], F32, name=_nm("ps_sB"), tag="mm")
                nc.tensor.matmul(psB, kTt[CH:P, h2, fc * P:(fc + 1) * P],
                                 qrhs[CH:P, h2, :], start=True, stop=True)
                nc.scalar.activation(out=expB[:, fc], in_=psB, func=AF.Exp,
                                     scale=SCALE)
                if eexp_dram is not None:
                    sync.dma_start(
                        out=eexp_dram[2 * h2, fc * P:(fc + 1) * P, :],
                        in_=expA[:, fc])
                    sync.dma_start(
                        out=eexp_dram[2 * h2 + 1, fc * P:(fc + 1) * P, :],
                        in_=expB[:, fc])
            psdA = den.tile([1, 512], F32, name=_nm("ps_dA"), tag="den")
            for fc in range(FCH):
                nc.tensor.matmul(psdA, ones128, expA[:, fc],
                                 start=(fc == 0), stop=(fc == FCH - 1))
            psdB = den.tile([1, 512], F32, name=_nm("ps_dB"), tag="den")
            for fc in range(FCH):
                nc.tensor.matmul(psdB, ones128, expB[:, fc],
                                 start=(fc == 0), stop=(fc == FCH - 1))
            psoA = acc.tile([CH, 512], F32, name=_nm("ps_oA"), tag="acc")
            for fc in range(FCH):
                nc.tensor.matmul(
                    psoA, vRt[:, fc, (2 * h2) * CH:(2 * h2 + 1) * CH],
                    expA[:, fc], start=(fc == 0), stop=(fc == FCH - 1))
            psoB = acc.tile([CH, 512], F32, name=_nm("ps_oB"), tag="acc")
            for fc in range(FCH):
                nc.tensor.matmul(
                    psoB, vRt[:, fc, (2 * h2 + 1) * CH:(2 * h2 + 2) * CH],
                    expB[:, fc], start=(fc == 0), stop=(fc == FCH - 1))
            for side, psd, pso in (("A", psdA, psoA), ("B", psdB, psoB)):
                rd = rbp.tile([1, 512], F32, name=_nm("rd"), tag="rd")
                nc.vector.reciprocal(out=rd, in_=psd)
                rdr = rbp.tile([1, 512], F32R, name=_nm("rdr"), tag="rdr")
                nc.vector.tensor_copy(out=rdr, in_=rd)
                psrb = mm.tile([P, 512], F32, name=_nm("ps_rb"), tag="mm")
                nc.tensor.matmul(psrb[0:CH], onesrow[:, 0:CH], rdr,
                                 start=True, stop=True)
                rb64 = rbp.tile([CH, 512], F32, name=_nm("rb64"), tag="rb64")
                nc.vector.tensor_copy(out=rb64, in_=psrb[0:CH])
                if side == "A":
                    nc.vector.tensor_mul(out=oT[0:CH, h2], in0=pso, in1=rb64)
                else:
                    tmpB = rbp.tile([CH, 512], F32R, name=_nm("tmpB"),
                                    tag="tmpB")
                    nc.vector.tensor_mul(out=tmpB, in0=pso, in1=rb64)
                    sync.dma_start(out=oT[CH:P, h2], in_=tmpB)

    # carriers that cross phase boundaries
    x1T = ctxstack.enter_context(tc.tile_pool(name="x1p", bufs=1)) \
        .tile([P, CC, R], F32R, name="x1T")

    # ============================ SA block ============================
    with tc.tile_pool(name="qkv", bufs=1) as qkv:
        kT = qkv.tile([P, CC, F], F32R, name="kT")
        qTt = qkv.tile([P, CC, R], F32R, name="qTt")
        vR = qkv.tile([P, CC, F], F32R, name="vR")

        with tc.tile_pool(name="saTmp", bufs=1) as saT, \
             tc.tile_pool(name="saW", bufs=2) as saW, \
             tc.tile_pool(name="saSq", bufs=2) as saSq, \
             tc.tile_pool(name="saStat", bufs=2) as saStat, \
             tc.tile_pool(name="saBc", bufs=1) as saBc:
            qTf = load_T(saT, t["qT"], CC, F, "qTf", tag="qTf")
            mbF, rbF = ln_stats(qTf, CC, F, "saF", saSq, saStat, saBc)
            ln_normalize(qTf, CC, mbF, rbF)
            proj_T(t["wk"], CC, CC, lambda k: qTf[:, k], F,
                   copy_out(kT, ck8), saW, 256, "w")
            # V row-major: lhsT = xn chunks, rhs = wv halves
            for n in range(2):
                nsl = slice(n * 512, (n + 1) * 512)
                wvt = saW.tile([P, CC, 512], F32R, name=_nm("wvt"), tag="wv",
                               bufs=1)
                for k in range(CC):
                    sync.dma_start(out=wvt[:, k],
                                   in_=t["wv"][k * P:(k + 1) * P, nsl])
                for fm in range(CC):
                    ps = mm.tile([P, 512], F32, name=_nm("ps_v"), tag="mm")
                    for k in range(CC):
                        nc.tensor.matmul(ps, qTf[:, k, fm * P:(fm + 1) * P],
                                         wvt[:, k],
                                         start=(k == 0), stop=(k == CC - 1))
                    nc.scalar.copy(out=vR[:, fm, nsl], in_=ps)
            qTl = load_T(saT, t["qTloc"], CC, R, "qTl", tag="qTf")
            mbL, rbL = ln_stats(qTl, CC, R, "saL", saSq, saStat, saBc)
            ln_normalize(qTl, CC, mbL, rbL)
            proj_T(t["wq"], CC, CC, lambda k: qTl[:, k], R,
                   copy_out(qTt, cq8), saW, 256, "w")

        with tc.tile_pool(name="saA", bufs=1) as expA_p, \
             tc.tile_pool(name="saB", bufs=1) as expB_p, \
             tc.tile_pool(name="saRb", bufs=2) as saRb, \
             tc.tile_pool(name="saO", bufs=1) as saO, \
             tc.tile_pool(name="saW2", bufs=2) as saW2, \
             tc.tile_pool(name="saRes", bufs=2) as saRes:
            oT = saO.tile([P, CC, R], F32R, name="oT")
            attn_heads(kT, qTt, vR, oT, expA_p, expB_p, saRb)

            qlr = t["qTloc"].rearrange("(ko ki) r -> ki ko r", ki=P)

            def sa_out(m, nsl, ps):
                res = saRes.tile([P, R], F32R, name=_nm("res"), tag="res")
                sync.dma_start(out=res, in_=qlr[:, m])
                nc.vector.scalar_tensor_tensor(
                    out=x1T[:, m], in0=ps, scalar=bo8[:, m:m + 1], in1=res,
                    op0=OP.add, op1=OP.add)

            proj_T(t["wo"], CC, CC, lambda k: oT[:, k], R, sa_out,
                   saW2, 256, "wo")

    # ============================ GRU block ============================
    if phases < 2:
        for m in range(CC):
            sync.dma_start(out=t["outT"][m * P:(m + 1) * P, :], in_=x1T[:, m])
            sync.dma_start(out=t["qhT"][m * P:(m + 1) * P, :], in_=x1T[:, m])
        for h in range(H):
            for fc in range(F // P):
                sync.dma_start(out=t["eexpT"][h, fc * P:(fc + 1) * P, :],
                               in_=x1T[:, fc % CC])
        return
    hpT = ctxstack.enter_context(tc.tile_pool(name="hpp", bufs=1)) \
        .tile([P, CC, R], F32R, name="hpT")
    with tc.tile_pool(name="gruT", bufs=1) as gT, \
         tc.tile_pool(name="gruW", bufs=2) as gW, \
         tc.tile_pool(name="gruSq", bufs=2) as gSq, \
         tc.tile_pool(name="gruStat", bufs=2) as gStat, \
         tc.tile_pool(name="gruBc", bufs=1) as gBc, \
         tc.tile_pool(name="gruTmp", bufs=2) as gTmp:
        hT = load_T(gT, t["hT"], CC, R, "hTt")
        xn2 = gT.tile([P, CC, R], F32R, name="xn2")
        for k in range(CC):
            nc.vector.tensor_copy(out=xn2[:, k], in_=x1T[:, k])
        mb2, rb2 = ln_stats(xn2, CC, R, "gru", gSq, gStat, gBc)
        ln_normalize(xn2, CC, mb2, rb2)

        rT = gT.tile([P, CC, R], F32R, name="rT")
        zT = gT.tile([P, CC, R], F32R, name="zT")
        for g in range(8):
            wt = gW.tile([P, 16, 256], F32R, name=_nm("wrzt"), tag="wrz")
            for k in range(16):
                sync.dma_start(out=wt[:, k],
                               in_=t["wrz"][k * P:(k + 1) * P,
                                            g * 256:(g + 1) * 256])
            for mi in range(2):
                m = g * 2 + mi
                ps = mm.tile([P, 512], F32, name=_nm("ps_rz"), tag="mm")
                for k in range(16):
                    rhs = xn2[:, k, :] if k < CC else hT[:, k - CC, :]
                    nc.tensor.matmul(ps, wt[:, k, mi * P:(mi + 1) * P], rhs,
                                     start=(k == 0), stop=(k == 15))
                dst = rT[:, m] if m < CC else zT[:, m - CC]
                nc.scalar.activation(out=dst, in_=ps, func=AF.Sigmoid,
                                     bias=brz16[:, m:m + 1])
        ghn = gT.tile([P, CC, R], F32R, name="ghn")
        proj_T(t["whn"], CC, CC, lambda k: hT[:, k], R,
               copy_out(ghn, bhn8), gW, 256, "wg")
        nT = rT  # n overwrites r per chunk (r is consumed first)

        def gin_out(m, nsl, ps):
            tmp = gTmp.tile([P, R], F32R, name=_nm("rghn"), tag="rghn")
            nc.vector.tensor_mul(out=tmp, in0=rT[:, m], in1=ghn[:, m])
            nc.vector.tensor_tensor(out=nT[:, m], in0=ps, in1=tmp, op=OP.add)
            nc.scalar.activation(out=nT[:, m], in_=nT[:, m], func=AF.Tanh,
                                 bias=bin8[:, m:m + 1])

        proj_T(t["win"], CC, CC, lambda k: xn2[:, k], R, gin_out,
               gW, 256, "wg")
        for m in range(CC):
            d = gTmp.tile([P, R], F32R, name=_nm("hmn"), tag="rghn")
            nc.vector.tensor_sub(out=d, in0=hT[:, m], in1=nT[:, m])
            nc.vector.tensor_mul(out=d, in0=zT[:, m], in1=d)
            nc.vector.tensor_add(out=hpT[:, m], in0=nT[:, m], in1=d)
            sync.dma_start(out=t["qhT"][m * P:(m + 1) * P, :], in_=hpT[:, m])

    # ============================ CA block ============================
    if phases < 3:
        for m in range(CC):
            sync.dma_start(out=t["outT"][m * P:(m + 1) * P, :], in_=x1T[:, m])
        for h in range(H):
            for fc in range(F // P):
                sync.dma_start(out=t["eexpT"][h, fc * P:(fc + 1) * P, :],
                               in_=x1T[:, fc % CC])
        return
    x2T = ctxstack.enter_context(tc.tile_pool(name="x2p", bufs=1)) \
        .tile([P, CC, R], F32R, name="x2T")
    with tc.tile_pool(name="caM", bufs=1) as caM:
        kkT = caM.tile([P, CC, F], F32R, name="kkT")
        v2R = caM.tile([P, CC, F], F32R, name="v2R")
        with tc.tile_pool(name="caCtx", bufs=1) as caC, \
             tc.tile_pool(name="caW", bufs=2) as caW:
            ctxT = load_T(caC, t["ctxT"], CC, F, "ctxTt")
            proj_T(t["wck"], CC, CC, lambda k: ctxT[:, k], F,
                   copy_out(kkT), caW, 256, "w")
            for n in range(2):
                nsl = slice(n * 512, (n + 1) * 512)
                wcvt = caW.tile([P, CC, 512], F32R, name=_nm("wcvt"),
                                tag="wv", bufs=1)
                for k in range(CC):
                    sync.dma_start(out=wcvt[:, k],
                                   in_=t["wcv"][k * P:(k + 1) * P, nsl])
                for fm in range(CC):
                    ps = mm.tile([P, 512], F32, name=_nm("ps_cv"), tag="mm")
                    for k in range(CC):
                        nc.tensor.matmul(ps, ctxT[:, k, fm * P:(fm + 1) * P],
                                         wcvt[:, k],
                                         start=(k == 0), stop=(k == CC - 1))
                    nc.scalar.copy(out=v2R[:, fm, nsl], in_=ps)

        with tc.tile_pool(name="caA", bufs=1) as cexpA, \
             tc.tile_pool(name="caB", bufs=1) as cexpB, \
             tc.tile_pool(name="caRb", bufs=2) as caRb, \
             tc.tile_pool(name="caO", bufs=1) as caO, \
             tc.tile_pool(name="caW2", bufs=2) as caW2:
            oT2 = caO.tile([P, CC, R], F32R, name="oT2")
            attn_heads(kkT, hpT, v2R, oT2, cexpA, cexpB, caRb,
                       eexp_dram=t["eexpT"])

            def ca_out(m, nsl, ps):
                nc.vector.scalar_tensor_tensor(
                    out=x2T[:, m], in0=ps, scalar=bco8[:, m:m + 1],
                    in1=x1T[:, m], op0=OP.add, op1=OP.add)

            proj_T(t["wco"], CC, CC, lambda k: oT2[:, k], R, ca_out,
                   caW2, 256, "wco")

    # ============================ MLP block ============================
    if phases < 4:
        for m in range(CC):
            sync.dma_start(out=t["outT"][m * P:(m + 1) * P, :], in_=x2T[:, m])
        return
    with tc.tile_pool(name="mlpT", bufs=1) as mT, \
         tc.tile_pool(name="mlpW", bufs=2) as mW, \
         tc.tile_pool(name="mlpSq", bufs=2) as mSq, \
         tc.tile_pool(name="mlpStat", bufs=2) as mStat, \
         tc.tile_pool(name="mlpBc", bufs=1) as mBc:
        s3 = mT.tile([P, CC, R], F32R, name="s3")
        for k in range(CC):
            nc.vector.tensor_copy(out=s3[:, k], in_=x2T[:, k])
        mb3, rb3 = ln_stats(s3, CC, R, "ln1", mSq, mStat, mBc)
        ln_normalize(s3, CC, mb3, rb3)
        for k in range(CC):
            nc.scalar.activation(out=s3[:, k], in_=s3[:, k], func=AF.Silu,
                                 scale=ln1w8[:, k:k + 1],
                                 bias=ln1b8[:, k:k + 1])
        m1 = mT.tile([P, 16, R], F32R, name="m1")
        proj_T(t["w1"], CC, 16, lambda k: s3[:, k], R,
               copy_out(m1), mW, 256, "w1")
        mb4, rb4 = ln_stats(m1, 16, R, "ln2", mSq, mStat, mBc)
        ln_normalize(m1, 16, mb4, rb4)
        for k in range(16):
            nc.scalar.activation(out=m1[:, k], in_=m1[:, k], func=AF.Silu,
                                 scale=ln2w16[:, k:k + 1],
                                 bias=ln2b16[:, k:k + 1])

        def mlp_out(m, nsl, ps):
            ot = mT.tile([P, R], F32R, name=_nm("otf"), tag="otf")
            nc.vector.tensor_tensor(out=ot, in0=ps, in1=x2T[:, m], op=OP.add)
            sync.dma_start(out=t["outT"][m * P:(m + 1) * P, :], in_=ot)

        proj_T(t["w2"], 16, CC, lambda k: m1[:, k], R, mlp_out,
               mW, 128, "w2")


def build_nc(phases=4):
    from contextlib import ExitStack
    nc = bacc.Bacc("TRN2", target_bir_lowering=False, debug=False)
    t = {}

    def inp(name, shape):
        t[name] = nc.dram_tensor(name, list(shape), F32R, kind="ExternalInput")

    inp("onesv", (P,))
    inp("qT", (C, F)); inp("qTloc", (C, R)); inp("hT", (L, R))
    inp("ctxT", (C, F))
    inp("wq", (C, L)); inp("wk", (C, L)); inp("wv", (C, C)); inp("wo", (C, C))
    inp("wrz", (2 * C, 2 * L)); inp("win", (C, L)); inp("whn", (L, L))
    inp("wck", (C, L)); inp("wcv", (C, C)); inp("wco", (C, C))
    inp("w1", (C, 2 * C)); inp("w2", (2 * C, C))
    for name, n in [("cq", C), ("ck", C), ("bo", C), ("brz", 2 * L),
                    ("bin", L), ("bhn", L), ("bco", C),
                    ("ln1w", C), ("ln1b", C), ("ln2w", 2 * C),
                    ("ln2b", 2 * C)]:
        t[name] = nc.dram_tensor(name, [n], F32, kind="ExternalInput")
    t["outT"] = nc.dram_tensor("outT", [C, R], F32R, kind="ExternalOutput")
    t["qhT"] = nc.dram_tensor("qhT", [L, R], F32R, kind="ExternalOutput")
    t["eexpT"] = nc.dram_tensor("eexpT", [H, KLEN, R], F32R,
                                kind="ExternalOutput")

    with tile.TileContext(nc) as tc, ExitStack() as ctxstack:
        _emit(nc, tc, t, ctxstack, phases)
    nc.compile()
    return nc


def prep_inputs(inputs):
    f = np.float32
    g = {k: np.asarray(v, f) for k, v in inputs.items()}
    qkvw = g["sa_qkv_w"].reshape(C, H, 2 * CH + C // H)
    Wq = np.ascontiguousarray(qkvw[:, :, :CH].reshape(C, L))
    Wk = np.ascontiguousarray(qkvw[:, :, CH:2 * CH].reshape(C, L))
    Wv = np.ascontiguousarray(qkvw[:, :, 2 * CH:].reshape(C, C))
    wln, bln = g["sa_norm_w"], g["sa_norm_b"]
    wq = wln[:, None] * Wq
    cq = bln @ Wq
    wk = wln[:, None] * Wk
    ck = bln @ Wk
    wv = wln[:, None] * Wv
    cv = bln @ Wv
    wo = g["sa_out_w"]
    bo = g["sa_out_b"] + cv @ wo

    W_ih, W_hh = g["gru_w_ih"], g["gru_w_hh"]
    b_ih, b_hh = g["gru_b_ih"], g["gru_b_hh"]
    cw, cb = g["ca_norm_w"], g["ca_norm_b"]
    ih_rzT = W_ih[:2 * L].T
    hh_rzT = W_hh[:2 * L].T
    wrz = np.concatenate([cw[:, None] * ih_rzT, hh_rzT], axis=0)
    brz = b_ih[:2 * L] + b_hh[:2 * L] + cb @ ih_rzT
    in_T = W_ih[2 * L:].T
    win = cw[:, None] * in_T
    bin_ = b_ih[2 * L:] + cb @ in_T
    whn = np.ascontiguousarray(W_hh[2 * L:].T)
    bhn = b_hh[2 * L:]

    shared = {
        "wq": wq, "wk": wk, "wv": wv, "wo": wo,
        "cq": cq, "ck": ck, "bo": bo,
        "wrz": wrz, "brz": brz, "win": win, "bin": bin_,
        "whn": whn, "bhn": bhn,
        "wck": g["ca_k_w"], "wcv": g["ca_v_w"], "wco": g["ca_out_w"],
        "bco": g["ca_out_b"],
        "w1": g["lin1_w"], "w2": g["lin2_w"],
        "ln1w": g["ln1_w"], "ln1b": g["ln1_b"],
        "ln2w": g["ln2_w"], "ln2b": g["ln2_b"],
    }
    shared = {k: np.ascontiguousarray(v, f) for k, v in shared.items()}

    in_maps = []
    for c in range(N_CORES):
        b = c // 2
        r0 = (c % 2) * R
        d = dict(shared)
        d["onesv"] = np.ones(128, np.float32)
        d["qT"] = np.ascontiguousarray(g["queries"][b].T)
        d["qTloc"] = np.ascontiguousarray(g["queries"][b, r0:r0 + R].T)
        d["hT"] = np.ascontiguousarray(
            g["queries_hidden"][b, r0:r0 + R].reshape(R, L).T)
        d["ctxT"] = np.ascontiguousarray(g["contexts"][b].T)
        in_maps.append(d)
    return in_maps


def unshard(results):
    out = np.empty((B, Q, C), np.float32)
    qh = np.empty((B, Q, H, CH), np.float32)
    energy = np.empty((B, H, Q, KLEN), np.float32)
    for c in range(N_CORES):
        b = c // 2
        r0 = (c % 2) * R
        res = results[c]
        out[b, r0:r0 + R] = res["outT"].T
        qh[b, r0:r0 + R] = res["qhT"].T.reshape(R, H, CH)
        with np.errstate(divide="ignore"):
            e = np.log(res["eexpT"].astype(np.float64)) / SCALE
        energy[b, :, r0:r0 + R, :] = e.transpose(0, 2, 1).astype(np.float32)
    return out, energy, qh


_NC_CACHE = None


def kernel(**inputs):
    global _NC_CACHE
    if _NC_CACHE is None:
        _NC_CACHE = build_nc()
    in_maps = prep_inputs(inputs)
    res = run_bass_kernel_spmd(_NC_CACHE, in_maps,
                               core_ids=list(range(N_CORES)))
    return unshard(res.results)
